# revision 1
# baseline (speedup 1.0000x reference)
"""Trainium2 Bass kernel for a cross-attention block.

reference semantics (jax):
    q = x @ Wq + bq                      # (b, hw, c)
    k = p @ Wk + bk                      # (b, 77, c)
    v = p @ Wv + bv                      # (b, 77, c)
    scores = einsum("bqhd,bkhd->bhqk", q, k) / sqrt(hd)
    attn = softmax(scores, -1)
    out = einsum("bhqk,bkhd->bqhd", attn, v) @ Ww + bw

Sharding: data-parallel over batch (16 batches / 8 cores = 2 per core),
no collectives.  Inside each core everything is computed in a
"features-on-partitions" (transposed) layout so that the contraction
dim of every matmul lands on SBUF partitions:

  X^T (via PE transpose)  ->  Q^T = Wq^T @ X^T
  scores^T[77, hw] = K^T_h.T @ Q^T_h            (per head, 2-head row packing)
  exp on ScalarE (scale=1/8 folded in, no max subtraction needed --
  |scores/8| < ~3 for this problem family)
  [num; den] = [V_h | 1]^T @ exp^T              (ones-augmented V matmul)
  attn_out^T = num * (1/den)                    (gpsimd partition_broadcast)
  out[hw,c]  = attn_out^T.T @ Ww  + bw          (natural layout -> contiguous store)

All matmuls run as float32r (fp32 bits, PE fast mode: 1 cycle/row when
the moving dim >= 256 vs 4 cycles/row for plain fp32).
"""

import numpy as np
from contextlib import ExitStack

import concourse.bass as bass
import concourse.tile as tile
from concourse import bacc, mybir
from concourse.bass_utils import run_bass_kernel_spmd
from concourse.masks import make_identity

N_CORES = 8
B_FULL, HW, C = 16, 4096, 1024
NH, D, CTX, NE = 16, 64, 77, 512
B = B_FULL // N_CORES          # batches per core
P = 128
KC = C // P                    # 8 c-chunks of 128
KN = NE // P                   # 4 n_embd chunks of 128
F = 256                        # hw elements per chunk
FSUB = F // P                  # 128-row subchunks per chunk

F32 = mybir.dt.float32
F32R = mybir.dt.float32r


def _r(ap):
    """Tag an fp32 AP as float32r for the PE fast path (same bits)."""
    return ap.bitcast(F32R)


def _bcast_dram(ap, parts, free):
    """DRAM 1-D tensor broadcast across `parts` partitions (step-0 AP)."""
    return bass.AP(tensor=ap.tensor, offset=ap.offset, ap=[[0, parts], [1, free]])


def _body(ctx: ExitStack, tc: tile.TileContext, io: dict, hw: int = HW):
    nc = tc.nc
    nchunk = hw // F

    x_ap, p_ap, out_ap = io["x"], io["p"], io["out"]
    wq_ap, bq_ap = io["Wq"], io["bq"]
    wk_ap, bk_ap = io["Wk"], io["bk"]
    wv_ap, bv_ap = io["Wv"], io["bv"]
    ww_ap, bw_ap = io["Ww"], io["bw"]

    # ---------------- pools ----------------
    # NOTE: pool address space is claimed in open order, so phase-B pools are
    # opened only after the phase-A scratch scope (wkv/ppool) closes.
    consts = ctx.enter_context(tc.tile_pool(name="consts", bufs=1))
    wpool = ctx.enter_context(tc.tile_pool(name="wpool", bufs=1))
    kvout = ctx.enter_context(tc.tile_pool(name="kvout", bufs=1))
    # PSUM: tags "tp"(1) + "qk"(2) + "at"(3) + "fin"(2) = 8 banks
    ps_tp = ctx.enter_context(tc.tile_pool(name="ps_tp", bufs=1, space="PSUM"))
    ps_qk = ctx.enter_context(tc.tile_pool(name="ps_qk", bufs=2, space="PSUM"))
    ps_at = ctx.enter_context(tc.tile_pool(name="ps_at", bufs=3, space="PSUM"))
    ps_fin = ctx.enter_context(tc.tile_pool(name="ps_fin", bufs=2, space="PSUM"))

    # ---------------- constants ----------------
    ident = consts.tile([P, P], F32, name="ident")
    make_identity(nc, ident[:])

    # per-cout-chunk bias columns: bq_sb[:, mc] == bq[mc*128 : (mc+1)*128]
    bq_sb = consts.tile([P, KC], F32, name="bq_sb")
    nc.sync.dma_start(out=bq_sb[:], in_=bq_ap.rearrange("(a b) -> b a", b=P))
    bk_sb = consts.tile([P, KC], F32, name="bk_sb")
    nc.sync.dma_start(out=bk_sb[:], in_=bk_ap.rearrange("(a b) -> b a", b=P))
    # free-dim biases broadcast across partitions (done once via DRAM DMA)
    bv_bc = consts.tile([CTX, C], F32, name="bv_bc")
    nc.sync.dma_start(out=bv_bc[:], in_=_bcast_dram(bv_ap, CTX, C))
    bw_bc = consts.tile([P, C], F32, name="bw_bc")
    nc.sync.dma_start(out=bw_bc[:], in_=_bcast_dram(bw_ap, P, C))

    # resident weights: Wq / Ww as 8 [128, 1024] k-slices (lhsT-ready)
    wq = []
    for k in range(KC):
        t = wpool.tile([P, C], F32R, name=f"wq{k}", tag=f"wq{k}")
        nc.sync.dma_start(out=t[:], in_=wq_ap[k * P : (k + 1) * P, :].bitcast(F32R))
        wq.append(t)
    ww = []
    for k in range(KC):
        t = wpool.tile([P, C], F32R, name=f"ww{k}", tag=f"ww{k}")
        nc.sync.dma_start(out=t[:], in_=ww_ap[k * P : (k + 1) * P, :].bitcast(F32R))
        ww.append(t)

    # K^T tiles [128, 77] per (batch, c-chunk); V augmented [77, NH, D+1]
    kT = [
        [kvout.tile([P, CTX], F32R, name=f"kT{b}_{m}", tag=f"kT{b}_{m}") for m in range(KC)]
        for b in range(B)
    ]
    v_aug = [
        kvout.tile([CTX, NH, D + 1], F32R, name=f"vaug{b}", tag=f"vaug{b}")
        for b in range(B)
    ]

    # ---------------- phase A: K/V projections (tiny) ----------------
    with ExitStack() as kvctx:
        wkv = kvctx.enter_context(tc.tile_pool(name="wkv", bufs=1))
        ppool = kvctx.enter_context(tc.tile_pool(name="ppool", bufs=2))
        wk = []
        wv = []
        for k in range(KN):
            t = wkv.tile([P, C], F32R, name=f"wk{k}", tag=f"wk{k}")
            nc.sync.dma_start(out=t[:], in_=wk_ap[k * P : (k + 1) * P, :].bitcast(F32R))
            wk.append(t)
            t = wkv.tile([P, C], F32R, name=f"wv{k}", tag=f"wv{k}")
            nc.sync.dma_start(out=t[:], in_=wv_ap[k * P : (k + 1) * P, :].bitcast(F32R))
            wv.append(t)

        for b in range(B):
            # p[b] natural [77, 512], then PE-transpose into pT [4][128, 77]
            pnat = ppool.tile([CTX, NE], F32, name="pnat", tag="pnat", bufs=2)
            nc.sync.dma_start(out=pnat[:], in_=p_ap[b])
            pT = []
            for k in range(KN):
                ps = ps_tp.tile([P, CTX], F32, name="ps_pT", tag="tp")
                nc.tensor.transpose(ps[:], pnat[:, k * P : (k + 1) * P], ident[:CTX, :CTX])
                t = ppool.tile([P, CTX], F32R, name=f"pT{k}", tag=f"pT{k}", bufs=2)
                nc.vector.tensor_copy(out=t[:], in_=ps[:])
                pT.append(t)

            # K^T[mc] = sum_k Wk[k,mc-slice].T @ pT[k]  (+ bk)
            for mc in range(KC):
                ps = ps_qk.tile([P, CTX], F32, name="ps_kT", tag="qk")
                for k in range(KN):
                    # N=77 is illegal for the fp32r fast path; plain fp32 here
                    # (tiny: 32 matmuls per batch).
                    nc.tensor.matmul(
                        ps[:],
                        wk[k][:, mc * P : (mc + 1) * P].bitcast(F32),
                        pT[k][:].bitcast(F32),
                        start=(k == 0),
                        stop=(k == KN - 1),
                    )
                nc.vector.tensor_add(
                    kT[b][mc][:], ps[:], bk_sb[:, mc : mc + 1].to_broadcast([P, CTX])
                )

            # V natural [77, c]: lhsT = pT[k] (K=128, M=77), rhs = Wv slice
            for nb in range(C // 512):
                ps = ps_at.tile([CTX, 512], F32, name="ps_v", tag="at")
                for k in range(KN):
                    nc.tensor.matmul(
                        ps[:],
                        pT[k][:],
                        wv[k][:, nb * 512 : (nb + 1) * 512],
                        start=(k == 0),
                        stop=(k == KN - 1),
                    )
                nc.vector.tensor_add(
                    v_aug[b][:, nb * 8 : (nb + 1) * 8, 0:D],
                    ps[:].rearrange("p (h d) -> p h d", d=D),
                    bv_bc[:, nb * 512 : (nb + 1) * 512].rearrange(
                        "p (h d) -> p h d", d=D
                    ),
                )
            # ones column for the fused softmax denominator
            nc.vector.memset(v_aug[b][:, :, D : D + 1].bitcast(F32), 1.0)

    # ---------------- phase B: main loop ----------------
    xpool = ctx.enter_context(tc.tile_pool(name="xpool", bufs=1))
    qpool = ctx.enter_context(tc.tile_pool(name="qpool", bufs=1))
    apool = ctx.enter_context(tc.tile_pool(name="apool", bufs=1))
    epool = ctx.enter_context(tc.tile_pool(name="epool", bufs=4))
    opool = ctx.enter_context(tc.tile_pool(name="opool", bufs=4))
    spool = ctx.enter_context(tc.tile_pool(name="spool", bufs=4))
    for b in range(B):
        for j in range(nchunk):
            r0 = j * F
            # x chunk natural [2][128, 1024]
            xn = []
            for r in range(FSUB):
                t = xpool.tile([P, C], F32, name="xn", tag="xn", bufs=4)
                nc.sync.dma_start(
                    out=t[:], in_=x_ap[b, r0 + r * P : r0 + (r + 1) * P, :]
                )
                xn.append(t)
            # PE-transpose -> xT[kc] [128(c), 256(hw)]
            xT = []
            for kc in range(KC):
                ps = ps_tp.tile([P, F], F32, name="ps_xT", tag="tp")
                for r in range(FSUB):
                    nc.tensor.transpose(
                        ps[:, r * P : (r + 1) * P],
                        xn[r][:, kc * P : (kc + 1) * P],
                        ident[:],
                    )
                t = xpool.tile([P, F], F32R, name="xT", tag="xT", bufs=16)
                nc.vector.tensor_copy(out=t[:], in_=ps[:])
                xT.append(t)

            # Q^T[mc] = sum_kc Wq[kc, mc-slice].T @ xT[kc]  (+ bq)
            qT = []
            for mc in range(KC):
                ps = ps_qk.tile([P, F], F32, name="ps_qT", tag="qk")
                for kc in range(KC):
                    nc.tensor.matmul(
                        ps[:],
                        wq[kc][:, mc * P : (mc + 1) * P],
                        xT[kc][:],
                        start=(kc == 0),
                        stop=(kc == KC - 1),
                    )
                t = qpool.tile([P, F], F32R, name="qT", tag="qT", bufs=16)
                nc.vector.tensor_add(
                    t[:], ps[:], bq_sb[:, mc : mc + 1].to_broadcast([P, F])
                )
                qT.append(t)

            # attention per head; attn-out^T accumulates into aT[kc][128, 256]
            aT = [
                apool.tile([P, F], F32R, name="aT", tag="aT", bufs=16)
                for _ in range(KC)
            ]
            for h in range(NH):
                mc, half = h // 2, (h % 2) * D
                # scores^T [77, F] = kT_h.T @ qT_h   (K = 64, row-packed pairs)
                ps_s = ps_at.tile([CTX, F], F32, name="ps_s", tag="at")
                nc.tensor.matmul(
                    ps_s[:],
                    kT[b][mc][half : half + D, :],
                    qT[mc][half : half + D, :],
                    start=True,
                    stop=True,
                    tile_position=(half, 0),
                )
                # exp(scores / 8) on ScalarE straight out of PSUM
                ex = epool.tile([CTX, F], F32R, name="ex", tag="ex")
                nc.scalar.activation(
                    ex[:], ps_s[:], mybir.ActivationFunctionType.Exp, scale=0.125
                )
                # [numerator; denominator] in one matmul via ones-augmented V
                ps_o = ps_at.tile([D + 1, F], F32, name="ps_o", tag="at")
                nc.tensor.matmul(
                    ps_o[:], v_aug[b][:, h, :], ex[:], start=True, stop=True
                )
                inv = spool.tile([1, F], F32, name="inv", tag="inv")
                nc.vector.reciprocal(out=inv[:], in_=ps_o[D : D + 1, :])
                bc = spool.tile([D, F], F32, name="bc", tag="bc")
                nc.gpsimd.partition_broadcast(bc[:], inv[:])
                nc.vector.tensor_mul(aT[mc][half : half + D, :], ps_o[0:D, :], bc[:])

            # final projection, natural orientation: out[hw128, c]
            for fs in range(FSUB):
                osb = opool.tile([P, C], F32, name="osb", tag="osb")
                for nb in range(C // 512):
                    ps = ps_fin.tile([P, 512], F32, name="ps_f", tag="fin")
                    for kc in range(KC):
                        nc.tensor.matmul(
                            ps[:],
                            aT[kc][:, fs * P : (fs + 1) * P],
                            ww[kc][:, nb * 512 : (nb + 1) * 512],
                            start=(kc == 0),
                            stop=(kc == KC - 1),
                        )
                    nc.vector.tensor_add(
                        osb[:, nb * 512 : (nb + 1) * 512],
                        ps[:],
                        bw_bc[:, nb * 512 : (nb + 1) * 512],
                    )
                nc.sync.dma_start(
                    out=out_ap[b, r0 + fs * P : r0 + (fs + 1) * P, :], in_=osb[:]
                )


def build_program(hw: int = HW):
    """Build + compile the per-core Bass program (SPMD, identical per core)."""
    nc = bacc.Bacc(
        "TRN2", target_bir_lowering=False, debug=False, num_devices=N_CORES
    )
    io = {}
    io["x"] = nc.dram_tensor("x", [B, hw, C], F32, kind="ExternalInput").ap()
    io["p"] = nc.dram_tensor("p", [B, CTX, NE], F32, kind="ExternalInput").ap()
    for name, shape in [
        ("Wq", [C, C]),
        ("bq", [C]),
        ("Wk", [NE, C]),
        ("bk", [C]),
        ("Wv", [NE, C]),
        ("bv", [C]),
        ("Ww", [C, C]),
        ("bw", [C]),
    ]:
        io[name] = nc.dram_tensor(name, shape, F32, kind="ExternalInput").ap()
    io["out"] = nc.dram_tensor("out", [B, hw, C], F32, kind="ExternalOutput").ap()

    with tile.TileContext(nc) as tc:
        with ExitStack() as ctx:
            _body(ctx, tc, io, hw=hw)
    nc.compile()
    return nc


_PROGRAM = None


def run_sharded(inputs: dict, trace: bool = False, **trace_kwargs):
    """Shard inputs over the 8 cores, run, gather. Returns (out, results)."""
    global _PROGRAM
    if _PROGRAM is None:
        _PROGRAM = build_program()
    nc = _PROGRAM

    full = {
        k: np.ascontiguousarray(v, dtype=np.float32)
        for k, v in inputs.items()
    }
    in_maps = []
    for i in range(N_CORES):
        m = dict(full)
        m["x"] = full["x"][i * B : (i + 1) * B]
        m["p"] = full["p"][i * B : (i + 1) * B]
        in_maps.append(m)

    res = run_bass_kernel_spmd(
        nc, in_maps, list(range(N_CORES)), trace=trace, **trace_kwargs
    )
    out = np.concatenate([res.results[i]["out"] for i in range(N_CORES)], axis=0)
    return out, res


def kernel(x, p, Wq, bq, Wk, bk, Wv, bv, Ww, bw):
    out, _ = run_sharded(
        dict(x=x, p=p, Wq=Wq, bq=bq, Wk=Wk, bk=bk, Wv=Wv, bv=bv, Ww=Ww, bw=bw)
    )
    return out



# revision 32
# speedup vs baseline: 1.3161x; 1.3161x over previous
"""Trainium2 Bass kernel for a cross-attention block.

reference semantics (jax):
    q = x @ Wq + bq                      # (b, hw, c)
    k = p @ Wk + bk                      # (b, 77, c)
    v = p @ Wv + bv                      # (b, 77, c)
    scores = einsum("bqhd,bkhd->bhqk", q, k) / sqrt(hd)
    attn = softmax(scores, -1)
    out = einsum("bhqk,bkhd->bqhd", attn, v) @ Ww + bw

Sharding: data-parallel over batch (16 batches / 8 cores = 2 per core),
no collectives.  Inside each core everything is computed in a
"features-on-partitions" (transposed) layout so that the contraction
dim of every matmul lands on SBUF partitions:

  X^T (via PE transpose)  ->  Q^T = Wq^T @ X^T
  scores^T[77, hw] = K^T_h.T @ Q^T_h            (per head, 2-head row packing)
  exp on ScalarE (scale=1/8 folded in, no max subtraction needed --
  |scores/8| < ~3 for this problem family)
  [num; den] = [V_h | 1]^T @ exp^T              (ones-augmented V matmul)
  attn_out^T = num * (1/den)                    (gpsimd partition_broadcast)
  out[hw,c]  = attn_out^T.T @ Ww  + bw          (natural layout -> contiguous store)

All matmuls run as float32r (fp32 bits, PE fast mode: 1 cycle/row when
the moving dim >= 256 vs 4 cycles/row for plain fp32).
"""

import numpy as np
from contextlib import ExitStack

import concourse.bass as bass
import concourse.tile as tile
from concourse import bacc, mybir
from concourse.bass_utils import run_bass_kernel_spmd
from concourse.masks import make_identity

N_CORES = 8
B_FULL, HW, C = 16, 4096, 1024
NH, D, CTX, NE = 16, 64, 77, 512
B = B_FULL // N_CORES          # batches per core
P = 128
KC = C // P                    # 8 c-chunks of 128
KN = NE // P                   # 4 n_embd chunks of 128
F = 256                        # hw elements per chunk
FSUB = F // P                  # 128-row subchunks per chunk

F32 = mybir.dt.float32
F32R = mybir.dt.float32r


def _r(ap):
    """Tag an fp32 AP as float32r for the PE fast path (same bits)."""
    return ap.bitcast(F32R)


def _bcast_dram(ap, parts, free):
    """DRAM 1-D tensor broadcast across `parts` partitions (step-0 AP)."""
    return bass.AP(tensor=ap.tensor, offset=ap.offset, ap=[[0, parts], [1, free]])


def _body(ctx: ExitStack, tc: tile.TileContext, io: dict, hw: int = HW):
    nc = tc.nc
    nchunk = hw // F

    x_ap, p_ap, out_ap = io["x"], io["p"], io["out"]
    wq_ap, bq_ap = io["Wq"], io["bq"]
    wk_ap, bk_ap = io["Wk"], io["bk"]
    wv_ap, bv_ap = io["Wv"], io["bv"]
    ww_ap, bw_ap = io["Ww"], io["bw"]

    # ---------------- pools ----------------
    # NOTE: pool address space is claimed in open order, so phase-B pools are
    # opened only after the phase-A scratch scope (wkv/ppool) closes.
    consts = ctx.enter_context(tc.tile_pool(name="consts", bufs=1))
    wpool = ctx.enter_context(tc.tile_pool(name="wpool", bufs=1))
    kvout = ctx.enter_context(tc.tile_pool(name="kvout", bufs=1))
    # PSUM banks (8): qk 2 (shared with transposes) + at 3 + den 1 + fin 2
    ps_qk = ctx.enter_context(tc.tile_pool(name="ps_qk", bufs=2, space="PSUM"))
    ps_at = ctx.enter_context(tc.tile_pool(name="ps_at", bufs=3, space="PSUM"))
    ps_den = ctx.enter_context(tc.tile_pool(name="ps_den", bufs=1, space="PSUM"))
    ps_fin = ctx.enter_context(tc.tile_pool(name="ps_fin", bufs=2, space="PSUM"))
    ps_tp = ps_qk

    # ---------------- constants ----------------
    ident = consts.tile([P, P], F32, name="ident")
    make_identity(nc, ident[:])

    # den-selector weights: slice [:, h, :] is [77, 128] with ones in column h
    # -> den matmul for head h accumulates its denominator into PSUM row h.
    # (free dim padded to 128 so the PE tile config stays 128x128)
    e_all = consts.tile([CTX, NH, P], F32R, name="e_all")
    # pair-broadcast weights: e2[p, q] = (p == q // 64); slice
    # [:, 128mc:128mc+128] maps inv_all rows (2mc, 2mc+1) onto output
    # partitions (0:64, 64:128) respectively. Rows >= 16 are all zero.
    e2_all = consts.tile([P, KC * P], F32R, name="e2_all")

    # per-cout-chunk bias columns: bq_sb[:, mc] == bq[mc*128 : (mc+1)*128]
    bq_sb = consts.tile([P, KC], F32, name="bq_sb")
    nc.sync.dma_start(out=bq_sb[:], in_=bq_ap.rearrange("(a b) -> b a", b=P))
    bk_sb = consts.tile([P, KC], F32, name="bk_sb")
    nc.sync.dma_start(out=bk_sb[:], in_=bk_ap.rearrange("(a b) -> b a", b=P))
    # free-dim biases broadcast across partitions (done once via DRAM DMA)
    bv_bc = consts.tile([CTX, C], F32, name="bv_bc")
    nc.sync.dma_start(out=bv_bc[:], in_=_bcast_dram(bv_ap, CTX, C))
    bw_bc = consts.tile([P, C], F32, name="bw_bc")
    nc.sync.dma_start(out=bw_bc[:], in_=_bcast_dram(bw_ap, P, C))

    # resident weights: Wq / Ww as 8 [128, 1024] k-slices (lhsT-ready)
    wq = []
    for k in range(KC):
        t = wpool.tile([P, C], F32R, name=f"wq{k}", tag=f"wq{k}")
        nc.sync.dma_start(out=t[:], in_=wq_ap[k * P : (k + 1) * P, :].bitcast(F32R))
        wq.append(t)
    ww = []
    for k in range(KC):
        t = wpool.tile([P, C], F32R, name=f"ww{k}", tag=f"ww{k}")
        nc.sync.dma_start(out=t[:], in_=ww_ap[k * P : (k + 1) * P, :].bitcast(F32R))
        ww.append(t)

    # K^T tiles [128, 77] per (batch, c-chunk); V zero-padded [77, NH, 128]:
    # head h occupies columns 64*(h%2) .. 64*(h%2)+64 of its slice so a head
    # pair accumulates into one full [128, F] PSUM tile with 128x128 tiles.
    kT = [
        [kvout.tile([P, CTX], F32R, name=f"kT{b}_{m}", tag=f"kT{b}_{m}") for m in range(KC)]
        for b in range(B)
    ]
    v_pad = [
        kvout.tile([CTX, NH, P], F32R, name=f"vpad{b}", tag=f"vpad{b}")
        for b in range(B)
    ]

    # ---------------- phase A: K/V projections (tiny) ----------------
    with ExitStack() as kvctx:
        wkv = kvctx.enter_context(tc.tile_pool(name="wkv", bufs=1))
        ppool = kvctx.enter_context(tc.tile_pool(name="ppool", bufs=2))
        scratch = kvctx.enter_context(tc.tile_pool(name="scratch", bufs=1))

        # gpsimd builds the selector patterns in fp32 scratch; DVE copies
        # produce the f32r-rounded tiles the fp32r matmuls require (the
        # gpsimd ucode can't take f32r APs).
        e_scr = scratch.tile([CTX, NH, P], F32, name="e_scr", tag="e_scr")
        nc.gpsimd.memset(e_scr[:], 1.0)
        nc.gpsimd.affine_select(
            out=e_scr[:],
            in_=e_scr[:],
            compare_op=mybir.AluOpType.is_equal,
            fill=0.0,
            base=0,
            pattern=[[1, NH], [-1, P]],  # keep only a == b (inner diagonal)
            channel_multiplier=0,
        )
        nc.vector.tensor_copy(out=e_all[:], in_=e_scr[:])
        e2_scr = scratch.tile([P, KC * P], F32, name="e2_scr", tag="e2_scr")
        nc.gpsimd.memset(e2_scr[:], 1.0)
        nc.gpsimd.affine_select(
            out=e2_scr[:],
            in_=e2_scr[:],
            compare_op=mybir.AluOpType.is_ge,
            fill=0.0,
            base=0,
            pattern=[[1, KC * P]],  # keep q - 64p >= 0
            channel_multiplier=-D,
        )
        nc.gpsimd.affine_select(
            out=e2_scr[:],
            in_=e2_scr[:],
            compare_op=mybir.AluOpType.is_ge,
            fill=0.0,
            base=D - 1,
            pattern=[[-1, KC * P]],  # keep 64p + 63 - q >= 0
            channel_multiplier=D,
        )
        nc.vector.tensor_copy(out=e2_all[:], in_=e2_scr[:])
        wk = []
        wv = []
        for k in range(KN):
            t = wkv.tile([P, C], F32R, name=f"wk{k}", tag=f"wk{k}")
            nc.sync.dma_start(out=t[:], in_=wk_ap[k * P : (k + 1) * P, :].bitcast(F32R))
            wk.append(t)
            t = wkv.tile([P, C], F32R, name=f"wv{k}", tag=f"wv{k}")
            nc.sync.dma_start(out=t[:], in_=wv_ap[k * P : (k + 1) * P, :].bitcast(F32R))
            wv.append(t)

        for b in range(B):
            # p[b] natural [77, 512], then PE-transpose into pT [4][128, 77]
            pnat = ppool.tile([CTX, NE], F32, name="pnat", tag="pnat", bufs=2)
            nc.sync.dma_start(out=pnat[:], in_=p_ap[b])
            pT = []
            for k in range(KN):
                ps = ps_tp.tile([P, CTX], F32, name="ps_pT", tag="qk")
                nc.tensor.transpose(ps[:], pnat[:, k * P : (k + 1) * P], ident[:CTX, :CTX])
                t = ppool.tile([P, CTX], F32R, name=f"pT{k}", tag=f"pT{k}", bufs=2)
                nc.vector.tensor_copy(out=t[:], in_=ps[:])
                pT.append(t)

            # K^T[mc] = sum_k Wk[k,mc-slice].T @ pT[k]  (+ bk)
            for mc in range(KC):
                ps = ps_qk.tile([P, CTX], F32, name="ps_kT", tag="qk")
                for k in range(KN):
                    # N=77 is illegal for the fp32r fast path; plain fp32 here
                    # (tiny: 32 matmuls per batch).
                    nc.tensor.matmul(
                        ps[:],
                        wk[k][:, mc * P : (mc + 1) * P].bitcast(F32),
                        pT[k][:].bitcast(F32),
                        start=(k == 0),
                        stop=(k == KN - 1),
                    )
                nc.vector.tensor_add(
                    kT[b][mc][:], ps[:], bk_sb[:, mc : mc + 1].to_broadcast([P, CTX])
                )

            # V natural [77, c]: lhsT = pT[k] (K=128, M=77), rhs = Wv slice
            nc.gpsimd.memset(v_pad[b][:].bitcast(F32), 0.0)
            vv = v_pad[b][:].rearrange("p (a two) c -> p a two c", two=2)
            for nb in range(C // 512):
                ps = ps_at.tile([CTX, 512], F32, name="ps_v", tag="at")
                for k in range(KN):
                    nc.tensor.matmul(
                        ps[:],
                        pT[k][:],
                        wv[k][:, nb * 512 : (nb + 1) * 512],
                        start=(k == 0),
                        stop=(k == KN - 1),
                    )
                pv = ps[:].rearrange("p (a two d) -> p a two d", two=2, d=D)
                bb = bv_bc[:, nb * 512 : (nb + 1) * 512].rearrange(
                    "p (a two d) -> p a two d", two=2, d=D
                )
                a0 = nb * 4
                nc.vector.tensor_add(
                    vv[:, a0 : a0 + 4, 0, 0:D], pv[:, :, 0, :], bb[:, :, 0, :]
                )
                nc.vector.tensor_add(
                    vv[:, a0 : a0 + 4, 1, D:P], pv[:, :, 1, :], bb[:, :, 1, :]
                )

    # ---------------- phase B: main loop ----------------
    xpool = ctx.enter_context(tc.tile_pool(name="xpool", bufs=1))
    qpool = ctx.enter_context(tc.tile_pool(name="qpool", bufs=1))
    apool = ctx.enter_context(tc.tile_pool(name="apool", bufs=1))
    epool = ctx.enter_context(tc.tile_pool(name="epool", bufs=4))
    opool = ctx.enter_context(tc.tile_pool(name="opool", bufs=4))
    spool = ctx.enter_context(tc.tile_pool(name="spool", bufs=4))
    for b in range(B):
        for j in range(nchunk):
            r0 = j * F
            # x chunk natural [2][128, 1024]
            xn = []
            for r in range(FSUB):
                t = xpool.tile([P, C], F32, name="xn", tag="xn", bufs=4)
                nc.sync.dma_start(
                    out=t[:], in_=x_ap[b, r0 + r * P : r0 + (r + 1) * P, :]
                )
                xn.append(t)
            # PE-transpose -> xT[kc] [128(c), 256(hw)]
            xT = []
            for kc in range(KC):
                ps = ps_tp.tile([P, F], F32, name="ps_xT", tag="qk")
                for r in range(FSUB):
                    nc.tensor.transpose(
                        ps[:, r * P : (r + 1) * P],
                        xn[r][:, kc * P : (kc + 1) * P],
                        ident[:],
                    )
                t = xpool.tile([P, F], F32R, name="xT", tag="xT", bufs=12)
                nc.scalar.copy(t[:], ps[:])
                xT.append(t)

            # Q^T[mc] = sum_kc Wq[kc, mc-slice].T @ xT[kc]  (+ bq)
            qT = []
            for mc in range(KC):
                ps = ps_qk.tile([P, F], F32, name="ps_qT", tag="qk")
                for kc in range(KC):
                    nc.tensor.matmul(
                        ps[:],
                        wq[kc][:, mc * P : (mc + 1) * P],
                        xT[kc][:],
                        start=(kc == 0),
                        stop=(kc == KC - 1),
                    )
                t = qpool.tile([P, F], F32R, name="qT", tag="qT", bufs=12)
                nc.scalar.add(t[:], ps[:], bq_sb[:, mc : mc + 1])
                qT.append(t)

            # attention: scores+exp pair-packed, batched denominators, then
            # per-pair AV + PE-broadcast + one [128, F] normalize mul.
            aT = [
                apool.tile([P, F], F32R, name="aT", tag="aT", bufs=16)
                for _ in range(KC)
            ]
            # scores per head into separate PSUM tiles (two independent
            # matmul groups must not share a PSUM bank on hw); exp lands in
            # the pair SBUF tile so den/av consume [77, 2, F] slices.
            exs = []
            for mc in range(KC):
                ex = epool.tile([CTX, 2, F], F32R, name="ex", tag="ex", bufs=8)
                for half in range(2):
                    ps_s = ps_at.tile([CTX, F], F32, name="ps_s", tag="at")
                    nc.tensor.matmul(
                        ps_s[:],
                        kT[b][mc][half * D : (half + 1) * D, :],
                        qT[mc][half * D : (half + 1) * D, :],
                        start=True,
                        stop=True,
                        tile_position=(half * D, 0),
                    )
                    nc.scalar.activation(
                        ex[:, half, :],
                        ps_s[:],
                        mybir.ActivationFunctionType.Exp,
                        scale=0.125,
                    )
                exs.append(ex)
            # all 16 denominators into rows 0:16 of one PSUM tile (accumulating
            # selector matmuls), then a single reciprocal for the chunk
            den = ps_den.tile([P, F], F32, name="den", tag="den")
            for h in range(NH):
                nc.tensor.matmul(
                    den[:],
                    e_all[:, h, :],
                    exs[h // 2][:, h % 2, :],
                    start=(h == 0),
                    stop=(h == NH - 1),
                )
            inv_all = spool.tile([P, F], F32R, name="inv_all", tag="inv")
            nc.gpsimd.memset(inv_all[:].bitcast(F32), 0.0)
            with nc.allow_low_precision(reason="inv feeds fp32r bcast matmul"):
                nc.vector.reciprocal(out=inv_all[0:NH, :], in_=den[0:NH, :])
            for mc in range(KC):
                # attn-out^T pair [128, F]: rows 0:64 head 2mc, 64:128 head 2mc+1
                # (zero-padded V slices -> both halves accumulate in one tile)
                ps_av = ps_at.tile([P, F], F32, name="ps_av", tag="at")
                for half in range(2):
                    h = 2 * mc + half
                    nc.tensor.matmul(
                        ps_av[:],
                        v_pad[b][:, h, :],
                        exs[mc][:, half, :],
                        start=(half == 0),
                        stop=(half == 1),
                    )
                # inv rows (2mc, 2mc+1) broadcast onto partitions via PE
                ps_bc = ps_at.tile([P, F], F32, name="ps_bc", tag="at")
                nc.tensor.matmul(
                    ps_bc[:],
                    e2_all[:, P * mc : P * (mc + 1)],
                    inv_all[:],
                    start=True,
                    stop=True,
                )
                bc = spool.tile([P, F], F32, name="bc", tag="bc")
                nc.scalar.copy(bc[:], ps_bc[:])
                nc.vector.tensor_mul(aT[mc][:], ps_av[:], bc[:])

            # final projection, natural orientation: out[hw128, c]
            for fs in range(FSUB):
                osb = opool.tile([P, C], F32, name="osb", tag="osb")
                for nb in range(C // 512):
                    ps = ps_fin.tile([P, 512], F32, name="ps_f", tag="fin")
                    for kc in range(KC):
                        nc.tensor.matmul(
                            ps[:],
                            aT[kc][:, fs * P : (fs + 1) * P],
                            ww[kc][:, nb * 512 : (nb + 1) * 512],
                            start=(kc == 0),
                            stop=(kc == KC - 1),
                        )
                    nc.vector.tensor_add(
                        osb[:, nb * 512 : (nb + 1) * 512],
                        ps[:],
                        bw_bc[:, nb * 512 : (nb + 1) * 512],
                    )
                nc.sync.dma_start(
                    out=out_ap[b, r0 + fs * P : r0 + (fs + 1) * P, :], in_=osb[:]
                )


def build_program(hw: int = HW):
    """Build + compile the per-core Bass program (SPMD, identical per core)."""
    nc = bacc.Bacc(
        "TRN2", target_bir_lowering=False, debug=False, num_devices=N_CORES
    )
    io = {}
    io["x"] = nc.dram_tensor("x", [B, hw, C], F32, kind="ExternalInput").ap()
    io["p"] = nc.dram_tensor("p", [B, CTX, NE], F32, kind="ExternalInput").ap()
    for name, shape in [
        ("Wq", [C, C]),
        ("bq", [C]),
        ("Wk", [NE, C]),
        ("bk", [C]),
        ("Wv", [NE, C]),
        ("bv", [C]),
        ("Ww", [C, C]),
        ("bw", [C]),
    ]:
        io[name] = nc.dram_tensor(name, shape, F32, kind="ExternalInput").ap()
    io["out"] = nc.dram_tensor("out", [B, hw, C], F32, kind="ExternalOutput").ap()

    with tile.TileContext(nc) as tc:
        with ExitStack() as ctx:
            _body(ctx, tc, io, hw=hw)
    nc.compile()
    return nc


_PROGRAM = None


def run_sharded(inputs: dict, trace: bool = False, **trace_kwargs):
    """Shard inputs over the 8 cores, run, gather. Returns (out, results)."""
    global _PROGRAM
    if _PROGRAM is None:
        _PROGRAM = build_program()
    nc = _PROGRAM

    full = {
        k: np.ascontiguousarray(v, dtype=np.float32)
        for k, v in inputs.items()
    }
    in_maps = []
    for i in range(N_CORES):
        m = dict(full)
        m["x"] = full["x"][i * B : (i + 1) * B]
        m["p"] = full["p"][i * B : (i + 1) * B]
        in_maps.append(m)

    res = run_bass_kernel_spmd(
        nc, in_maps, list(range(N_CORES)), trace=trace, **trace_kwargs
    )
    out = np.concatenate([res.results[i]["out"] for i in range(N_CORES)], axis=0)
    return out, res


def kernel(x, p, Wq, bq, Wk, bk, Wv, bv, Ww, bw):
    out, _ = run_sharded(
        dict(x=x, p=p, Wq=Wq, bq=bq, Wk=Wk, bk=bk, Wv=Wv, bv=bv, Ww=Ww, bw=bw)
    )
    return out



# revision 33
# speedup vs baseline: 1.5961x; 1.2127x over previous
"""Trainium2 Bass kernel for a cross-attention block.

reference semantics (jax):
    q = x @ Wq + bq                      # (b, hw, c)
    k = p @ Wk + bk                      # (b, 77, c)
    v = p @ Wv + bv                      # (b, 77, c)
    scores = einsum("bqhd,bkhd->bhqk", q, k) / sqrt(hd)
    attn = softmax(scores, -1)
    out = einsum("bhqk,bkhd->bqhd", attn, v) @ Ww + bw

Sharding: data-parallel over batch (16 batches / 8 cores = 2 per core),
no collectives.  Inside each core everything is computed in a
"features-on-partitions" (transposed) layout so that the contraction
dim of every matmul lands on SBUF partitions:

  X^T (via PE transpose)  ->  Q^T = Wq^T @ X^T      (bf16 weights/acts)
  scores^T[77, hw] = K^T_h.T @ Q^T_h                (per head)
  exp on ScalarE (scale=1/8 folded in, no max subtraction needed --
  |scores/8| < ~3 for this problem family)
  denominators: 16 selector matmuls accumulate all head sums into one
  [16, F] PSUM tile -> ONE DVE reciprocal per chunk
  broadcast: tiny PE matmul maps inv rows onto 128 partitions per pair
  attn_out^T pair = ps_av * bc                      (one [128, F] DVE mul)
  out[hw,c]  = attn_out^T.T @ Ww  + bw              (natural layout store)

Matmuls run in bf16 (1 cycle/row, half-size LDWEIGHTS - the weight-load
queue is the limiter at fp32) with fp32 PSUM accumulation.  The inv
broadcast path stays fp32r for precision of 1/den.

Hardware pitfalls encoded here:
  - two independent matmul groups must not share a PSUM bank
  - PE tile configs must stay 128x128 (pad selector weights/outputs)
  - gpsimd ucode takes fp32 APs only (build consts in fp32, cast on DVE)
  - DVE tensor ops read at most one PSUM operand
"""

import numpy as np
from contextlib import ExitStack

import concourse.bass as bass
import concourse.tile as tile
from concourse import bacc, mybir
from concourse.bass_utils import run_bass_kernel_spmd
from concourse.masks import make_identity

N_CORES = 8
B_FULL, HW, C = 16, 4096, 1024
NH, D, CTX, NE = 16, 64, 77, 512
B = B_FULL // N_CORES          # batches per core
P = 128
KC = C // P                    # 8 c-chunks of 128
KN = NE // P                   # 4 n_embd chunks of 128
F = 512                        # hw elements per chunk
FSUB = F // P                  # 128-row subchunks per chunk

F32 = mybir.dt.float32
F32R = mybir.dt.float32r
BF16 = mybir.dt.bfloat16


def _bcast_dram(ap, parts, free):
    """DRAM 1-D tensor broadcast across `parts` partitions (step-0 AP)."""
    return bass.AP(tensor=ap.tensor, offset=ap.offset, ap=[[0, parts], [1, free]])


def _body(ctx: ExitStack, tc: tile.TileContext, io: dict, hw: int = HW):
    nc = tc.nc
    nchunk = hw // F

    x_ap, p_ap, out_ap = io["x"], io["p"], io["out"]
    wq_ap, bq_ap = io["Wq"], io["bq"]
    wk_ap, bk_ap = io["Wk"], io["bk"]
    wv_ap, bv_ap = io["Wv"], io["bv"]
    ww_ap, bw_ap = io["Ww"], io["bw"]

    # ---------------- pools ----------------
    # NOTE: pool address space is claimed in open order, so phase-B pools are
    # opened only after the phase-A scratch scope (wkv/ppool/scratch) closes.
    consts = ctx.enter_context(tc.tile_pool(name="consts", bufs=1))
    wpool = ctx.enter_context(tc.tile_pool(name="wpool", bufs=1))
    kvout = ctx.enter_context(tc.tile_pool(name="kvout", bufs=1))
    # PSUM banks (8): qk 2 (shared with transposes) + at 3 + den 1 + fin 2
    ps_qk = ctx.enter_context(tc.tile_pool(name="ps_qk", bufs=2, space="PSUM"))
    ps_at = ctx.enter_context(tc.tile_pool(name="ps_at", bufs=3, space="PSUM"))
    ps_den = ctx.enter_context(tc.tile_pool(name="ps_den", bufs=1, space="PSUM"))
    ps_fin = ctx.enter_context(tc.tile_pool(name="ps_fin", bufs=2, space="PSUM"))
    ps_tp = ps_qk

    # ---------------- constants ----------------
    ident = consts.tile([P, P], F32, name="ident")
    make_identity(nc, ident[:])

    # den-selector weights: slice [:, h, :] is [77, 128] with ones in column h
    # -> den matmul for head h accumulates its denominator into PSUM row h.
    # (free dim padded to 128 so the PE tile config stays 128x128)
    e_all = consts.tile([CTX, NH, P], BF16, name="e_all")
    # pair-broadcast weights: e2[p, q] = (p == q // 64); slice
    # [:, 128mc:128mc+128] maps inv_all rows (2mc, 2mc+1) onto output
    # partitions (0:64, 64:128) respectively. Rows >= 16 are all zero.
    e2_all = consts.tile([P, KC * P], F32R, name="e2_all")

    # per-cout-chunk bias columns: bq_sb[:, mc] == bq[mc*128 : (mc+1)*128]
    bq_sb = consts.tile([P, KC], F32, name="bq_sb")
    nc.sync.dma_start(out=bq_sb[:], in_=bq_ap.rearrange("(a b) -> b a", b=P))
    bk_sb = consts.tile([P, KC], F32, name="bk_sb")
    nc.sync.dma_start(out=bk_sb[:], in_=bk_ap.rearrange("(a b) -> b a", b=P))
    # bw broadcast across partitions (done once via DRAM DMA)
    bw_bc = consts.tile([P, C], F32, name="bw_bc")
    nc.sync.dma_start(out=bw_bc[:], in_=_bcast_dram(bw_ap, P, C))

    # resident weights: Wq / Ww as 8 bf16 [128, 1024] k-slices (lhsT-ready)
    wq = [wpool.tile([P, C], BF16, name=f"wq{k}", tag=f"wq{k}") for k in range(KC)]
    ww = [wpool.tile([P, C], BF16, name=f"ww{k}", tag=f"ww{k}") for k in range(KC)]

    # K^T tiles [128, 77] per (batch, c-chunk); V zero-padded [77, NH, 128]:
    # head h occupies columns 64*(h%2) .. 64*(h%2)+64 of its slice so a head
    # pair accumulates into one full [128, F] PSUM tile with 128x128 tiles.
    kT = [
        [kvout.tile([P, CTX], BF16, name=f"kT{b}_{m}", tag=f"kT{b}_{m}") for m in range(KC)]
        for b in range(B)
    ]
    v_pad = [
        kvout.tile([CTX, NH, P], BF16, name=f"vpad{b}", tag=f"vpad{b}")
        for b in range(B)
    ]

    # ---------------- phase A: consts, weight casts, K/V projections ---------
    with ExitStack() as kvctx:
        wkv = kvctx.enter_context(tc.tile_pool(name="wkv", bufs=1))
        ppool = kvctx.enter_context(tc.tile_pool(name="ppool", bufs=2))
        scratch = kvctx.enter_context(tc.tile_pool(name="scratch", bufs=1))

        # gpsimd builds the selector patterns in fp32 scratch; DVE copies
        # produce the rounded tiles the matmuls require (the gpsimd ucode
        # can't take f32r/bf16 APs).
        e_scr = scratch.tile([CTX, NH, P], F32, name="e_scr", tag="e_scr")
        nc.gpsimd.memset(e_scr[:], 1.0)
        nc.gpsimd.affine_select(
            out=e_scr[:],
            in_=e_scr[:],
            compare_op=mybir.AluOpType.is_equal,
            fill=0.0,
            base=0,
            pattern=[[1, NH], [-1, P]],  # keep only a == b (inner diagonal)
            channel_multiplier=0,
        )
        e2_scr = scratch.tile([P, KC * P], F32, name="e2_scr", tag="e2_scr")
        nc.gpsimd.memset(e2_scr[:], 1.0)
        nc.gpsimd.affine_select(
            out=e2_scr[:],
            in_=e2_scr[:],
            compare_op=mybir.AluOpType.is_ge,
            fill=0.0,
            base=0,
            pattern=[[1, KC * P]],  # keep q - 64p >= 0
            channel_multiplier=-D,
        )
        nc.gpsimd.affine_select(
            out=e2_scr[:],
            in_=e2_scr[:],
            compare_op=mybir.AluOpType.is_ge,
            fill=0.0,
            base=D - 1,
            pattern=[[-1, KC * P]],  # keep 64p + 63 - q >= 0
            channel_multiplier=D,
        )
        with nc.allow_low_precision(reason="0/1 selector weights exact in bf16"):
            nc.vector.tensor_copy(out=e_all[:], in_=e_scr[:])
            nc.vector.tensor_copy(out=e2_all[:], in_=e2_scr[:])

        # cast Wq / Ww to resident bf16 tiles through a fp32 staging buffer
        with nc.allow_low_precision(reason="bf16 weights, fp32 PSUM accum"):
            for k in range(KC):
                st = scratch.tile([P, C], F32, name="wstage", tag="wstage", bufs=2)
                nc.sync.dma_start(out=st[:], in_=wq_ap[k * P : (k + 1) * P, :])
                nc.vector.tensor_copy(out=wq[k][:], in_=st[:])
                st = scratch.tile([P, C], F32, name="wstage", tag="wstage", bufs=2)
                nc.sync.dma_start(out=st[:], in_=ww_ap[k * P : (k + 1) * P, :])
                nc.vector.tensor_copy(out=ww[k][:], in_=st[:])

        wk = []
        wv = []
        for k in range(KN):
            t = wkv.tile([P, C], F32R, name=f"wk{k}", tag=f"wk{k}")
            nc.sync.dma_start(out=t[:], in_=wk_ap[k * P : (k + 1) * P, :].bitcast(F32R))
            wk.append(t)
            t = wkv.tile([P, C], F32R, name=f"wv{k}", tag=f"wv{k}")
            nc.sync.dma_start(out=t[:], in_=wv_ap[k * P : (k + 1) * P, :].bitcast(F32R))
            wv.append(t)
        # bv broadcast across partitions, only needed during phase A
        bv_bc = scratch.tile([CTX, C], F32, name="bv_bc", tag="bv_bc")
        nc.sync.dma_start(out=bv_bc[:], in_=_bcast_dram(bv_ap, CTX, C))

        for b in range(B):
            # p[b] natural [77, 512], then PE-transpose into pT [4][128, 77]
            pnat = ppool.tile([CTX, NE], F32, name="pnat", tag="pnat", bufs=2)
            nc.sync.dma_start(out=pnat[:], in_=p_ap[b])
            pT = []
            for k in range(KN):
                ps = ps_tp.tile([P, CTX], F32, name="ps_pT", tag="qk")
                nc.tensor.transpose(ps[:], pnat[:, k * P : (k + 1) * P], ident[:CTX, :CTX])
                t = ppool.tile([P, CTX], F32R, name=f"pT{k}", tag=f"pT{k}", bufs=2)
                nc.vector.tensor_copy(out=t[:], in_=ps[:])
                pT.append(t)

            with nc.allow_low_precision(reason="bf16 K/V, fp32 PSUM accum"):
                # K^T[mc] = sum_k Wk[k,mc-slice].T @ pT[k]  (+ bk)
                for mc in range(KC):
                    ps = ps_qk.tile([P, CTX], F32, name="ps_kT", tag="qk")
                    for k in range(KN):
                        # N=77 is illegal for the fp32r fast path; plain fp32
                        # here (tiny: 32 matmuls per batch).
                        nc.tensor.matmul(
                            ps[:],
                            wk[k][:, mc * P : (mc + 1) * P].bitcast(F32),
                            pT[k][:].bitcast(F32),
                            start=(k == 0),
                            stop=(k == KN - 1),
                        )
                    nc.vector.tensor_add(
                        kT[b][mc][:], ps[:], bk_sb[:, mc : mc + 1].to_broadcast([P, CTX])
                    )

                # V natural [77, c]: lhsT = pT[k] (K=128, M=77), rhs = Wv slice
                nc.gpsimd.memset(v_pad[b][:].bitcast(mybir.dt.uint16), 0.0)
                vv = v_pad[b][:].rearrange("p (a two) c -> p a two c", two=2)
                for nb in range(C // 512):
                    ps = ps_at.tile([CTX, 512], F32, name="ps_v", tag="at")
                    for k in range(KN):
                        nc.tensor.matmul(
                            ps[:],
                            pT[k][:],
                            wv[k][:, nb * 512 : (nb + 1) * 512],
                            start=(k == 0),
                            stop=(k == KN - 1),
                        )
                    pv = ps[:].rearrange("p (a two d) -> p a two d", two=2, d=D)
                    bb = bv_bc[:, nb * 512 : (nb + 1) * 512].rearrange(
                        "p (a two d) -> p a two d", two=2, d=D
                    )
                    a0 = nb * 4
                    nc.vector.tensor_add(
                        vv[:, a0 : a0 + 4, 0, 0:D], pv[:, :, 0, :], bb[:, :, 0, :]
                    )
                    nc.vector.tensor_add(
                        vv[:, a0 : a0 + 4, 1, D:P], pv[:, :, 1, :], bb[:, :, 1, :]
                    )

    # ---------------- phase B: main loop ----------------
    xpool = ctx.enter_context(tc.tile_pool(name="xpool", bufs=1))
    qpool = ctx.enter_context(tc.tile_pool(name="qpool", bufs=1))
    apool = ctx.enter_context(tc.tile_pool(name="apool", bufs=1))
    epool = ctx.enter_context(tc.tile_pool(name="epool", bufs=4))
    opool = ctx.enter_context(tc.tile_pool(name="opool", bufs=4))
    spool = ctx.enter_context(tc.tile_pool(name="spool", bufs=4))
    ctx.enter_context(
        nc.allow_low_precision(reason="bf16 activations, fp32 PSUM accum")
    )
    for b in range(B):
        for j in range(nchunk):
            r0 = j * F
            # x chunk natural [4][128, 1024]
            xn = []
            for r in range(FSUB):
                t = xpool.tile([P, C], F32, name="xn", tag="xn", bufs=6)
                nc.sync.dma_start(
                    out=t[:], in_=x_ap[b, r0 + r * P : r0 + (r + 1) * P, :]
                )
                xn.append(t)
            # PE-transpose -> xT[kc] [128(c), 512(hw)] in bf16
            xT = []
            for kc in range(KC):
                ps = ps_tp.tile([P, F], F32, name="ps_xT", tag="qk")
                for r in range(FSUB):
                    nc.tensor.transpose(
                        ps[:, r * P : (r + 1) * P],
                        xn[r][:, kc * P : (kc + 1) * P],
                        ident[:],
                    )
                t = xpool.tile([P, F], BF16, name="xT", tag="xT", bufs=10)
                nc.scalar.copy(t[:], ps[:])
                xT.append(t)

            # Q^T[mc] = sum_kc Wq[kc, mc-slice].T @ xT[kc]  (+ bq)
            qT = []
            for mc in range(KC):
                ps = ps_qk.tile([P, F], F32, name="ps_qT", tag="qk")
                for kc in range(KC):
                    nc.tensor.matmul(
                        ps[:],
                        wq[kc][:, mc * P : (mc + 1) * P],
                        xT[kc][:],
                        start=(kc == 0),
                        stop=(kc == KC - 1),
                    )
                t = qpool.tile([P, F], BF16, name="qT", tag="qT", bufs=10)
                nc.scalar.add(t[:], ps[:], bq_sb[:, mc : mc + 1])
                qT.append(t)

            # attention: per-head scores+exp, batched denominators, then
            # per-pair AV + PE-broadcast + one [128, F] normalize mul.
            aT = [
                apool.tile([P, F], BF16, name="aT", tag="aT", bufs=10)
                for _ in range(KC)
            ]
            # scores per head into separate PSUM tiles (two independent
            # matmul groups must not share a PSUM bank on hw); exp lands in
            # the pair SBUF tile so den/av consume [77, 2, F] slices.
            exs = []
            for mc in range(KC):
                ex = epool.tile([CTX, 2, F], BF16, name="ex", tag="ex", bufs=8)
                for half in range(2):
                    ps_s = ps_at.tile([CTX, F], F32, name="ps_s", tag="at")
                    nc.tensor.matmul(
                        ps_s[:],
                        kT[b][mc][half * D : (half + 1) * D, :],
                        qT[mc][half * D : (half + 1) * D, :],
                        start=True,
                        stop=True,
                        tile_position=(half * D, 0),
                    )
                    nc.scalar.activation(
                        ex[:, half, :],
                        ps_s[:],
                        mybir.ActivationFunctionType.Exp,
                        scale=0.125,
                    )
                exs.append(ex)
            # all 16 denominators into rows 0:16 of one PSUM tile (accumulating
            # selector matmuls), then a single reciprocal for the chunk
            den = ps_den.tile([P, F], F32, name="den", tag="den")
            for h in range(NH):
                nc.tensor.matmul(
                    den[:],
                    e_all[:, h, :],
                    exs[h // 2][:, h % 2, :],
                    start=(h == 0),
                    stop=(h == NH - 1),
                )
            inv_all = spool.tile([P, F], F32R, name="inv_all", tag="inv", bufs=2)
            nc.gpsimd.memset(inv_all[:].bitcast(F32), 0.0)
            nc.vector.reciprocal(out=inv_all[0:NH, :], in_=den[0:NH, :])
            for mc in range(KC):
                # attn-out^T pair [128, F]: rows 0:64 head 2mc, 64:128 head 2mc+1
                # (zero-padded V slices -> both halves accumulate in one tile)
                ps_av = ps_at.tile([P, F], F32, name="ps_av", tag="at")
                for half in range(2):
                    h = 2 * mc + half
                    nc.tensor.matmul(
                        ps_av[:],
                        v_pad[b][:, h, :],
                        exs[mc][:, half, :],
                        start=(half == 0),
                        stop=(half == 1),
                    )
                # inv rows (2mc, 2mc+1) broadcast onto partitions via PE
                ps_bc = ps_at.tile([P, F], F32, name="ps_bc", tag="at")
                nc.tensor.matmul(
                    ps_bc[:],
                    e2_all[:, P * mc : P * (mc + 1)],
                    inv_all[:],
                    start=True,
                    stop=True,
                )
                bc = spool.tile([P, F], F32, name="bc", tag="bc")
                nc.scalar.copy(bc[:], ps_bc[:])
                nc.vector.tensor_mul(aT[mc][:], ps_av[:], bc[:])

            # final projection, natural orientation: out[hw128, c]
            for fs in range(FSUB):
                osb = opool.tile([P, C], F32, name="osb", tag="osb", bufs=3)
                for nb in range(C // 512):
                    ps = ps_fin.tile([P, 512], F32, name="ps_f", tag="fin")
                    for kc in range(KC):
                        nc.tensor.matmul(
                            ps[:],
                            aT[kc][:, fs * P : (fs + 1) * P],
                            ww[kc][:, nb * 512 : (nb + 1) * 512],
                            start=(kc == 0),
                            stop=(kc == KC - 1),
                        )
                    nc.vector.tensor_add(
                        osb[:, nb * 512 : (nb + 1) * 512],
                        ps[:],
                        bw_bc[:, nb * 512 : (nb + 1) * 512],
                    )
                nc.sync.dma_start(
                    out=out_ap[b, r0 + fs * P : r0 + (fs + 1) * P, :], in_=osb[:]
                )


def build_program(hw: int = HW):
    """Build + compile the per-core Bass program (SPMD, identical per core)."""
    nc = bacc.Bacc(
        "TRN2", target_bir_lowering=False, debug=False, num_devices=N_CORES
    )
    io = {}
    io["x"] = nc.dram_tensor("x", [B, hw, C], F32, kind="ExternalInput").ap()
    io["p"] = nc.dram_tensor("p", [B, CTX, NE], F32, kind="ExternalInput").ap()
    for name, shape in [
        ("Wq", [C, C]),
        ("bq", [C]),
        ("Wk", [NE, C]),
        ("bk", [C]),
        ("Wv", [NE, C]),
        ("bv", [C]),
        ("Ww", [C, C]),
        ("bw", [C]),
    ]:
        io[name] = nc.dram_tensor(name, shape, F32, kind="ExternalInput").ap()
    io["out"] = nc.dram_tensor("out", [B, hw, C], F32, kind="ExternalOutput").ap()

    with tile.TileContext(nc) as tc:
        with ExitStack() as ctx:
            _body(ctx, tc, io, hw=hw)
    nc.compile()
    return nc


_PROGRAM = None


def run_sharded(inputs: dict, trace: bool = False, **trace_kwargs):
    """Shard inputs over the 8 cores, run, gather. Returns (out, results)."""
    global _PROGRAM
    if _PROGRAM is None:
        _PROGRAM = build_program()
    nc = _PROGRAM

    full = {
        k: np.ascontiguousarray(v, dtype=np.float32)
        for k, v in inputs.items()
    }
    in_maps = []
    for i in range(N_CORES):
        m = dict(full)
        m["x"] = full["x"][i * B : (i + 1) * B]
        m["p"] = full["p"][i * B : (i + 1) * B]
        in_maps.append(m)

    res = run_bass_kernel_spmd(
        nc, in_maps, list(range(N_CORES)), trace=trace, **trace_kwargs
    )
    out = np.concatenate([res.results[i]["out"] for i in range(N_CORES)], axis=0)
    return out, res


def kernel(x, p, Wq, bq, Wk, bk, Wv, bv, Ww, bw):
    out, _ = run_sharded(
        dict(x=x, p=p, Wq=Wq, bq=bq, Wk=Wk, bk=bk, Wv=Wv, bv=bv, Ww=Ww, bw=bw)
    )
    return out


# revision 34
# speedup vs baseline: 1.7357x; 1.0875x over previous
"""Trainium2 Bass kernel for a cross-attention block.

reference semantics (jax):
    q = x @ Wq + bq                      # (b, hw, c)
    k = p @ Wk + bk                      # (b, 77, c)
    v = p @ Wv + bv                      # (b, 77, c)
    scores = einsum("bqhd,bkhd->bhqk", q, k) / sqrt(hd)
    attn = softmax(scores, -1)
    out = einsum("bhqk,bkhd->bqhd", attn, v) @ Ww + bw

Sharding: data-parallel over batch (16 batches / 8 cores = 2 per core),
no collectives.  Inside each core everything is computed in a
"features-on-partitions" (transposed) layout so that the contraction
dim of every matmul lands on SBUF partitions:

  X^T (via PE transpose)  ->  Q^T = Wq^T @ X^T      (bf16 weights/acts)
  scores^T[77, hw] = K^T_h.T @ Q^T_h                (per head)
  exp on ScalarE (scale=1/8 folded in, no max subtraction needed --
  |scores/8| < ~3 for this problem family)
  denominators: 16 selector matmuls accumulate all head sums into one
  [16, F] PSUM tile -> ONE DVE reciprocal per chunk
  broadcast: tiny PE matmul maps inv rows onto 128 partitions per pair
  attn_out^T pair = ps_av * bc                      (one [128, F] DVE mul)
  out[hw,c]  = attn_out^T.T @ Ww  + bw              (natural layout store)

Matmuls run in bf16 (1 cycle/row, half-size LDWEIGHTS - the weight-load
queue is the limiter at fp32) with fp32 PSUM accumulation.  The inv
broadcast path stays fp32r for precision of 1/den.

Hardware pitfalls encoded here:
  - two independent matmul groups must not share a PSUM bank
  - PE tile configs must stay 128x128 (pad selector weights/outputs)
  - gpsimd ucode takes fp32 APs only (build consts in fp32, cast on DVE)
  - DVE tensor ops read at most one PSUM operand
"""

import numpy as np
from contextlib import ExitStack

import concourse.bass as bass
import concourse.tile as tile
from concourse import bacc, mybir
from concourse.bass_utils import run_bass_kernel_spmd
from concourse.masks import make_identity

N_CORES = 8
B_FULL, HW, C = 16, 4096, 1024
NH, D, CTX, NE = 16, 64, 77, 512
B = B_FULL // N_CORES          # batches per core
P = 128
KC = C // P                    # 8 c-chunks of 128
KN = NE // P                   # 4 n_embd chunks of 128
F = 512                        # hw elements per chunk
FSUB = F // P                  # 128-row subchunks per chunk

F32 = mybir.dt.float32
F32R = mybir.dt.float32r
BF16 = mybir.dt.bfloat16


def _bcast_dram(ap, parts, free):
    """DRAM 1-D tensor broadcast across `parts` partitions (step-0 AP)."""
    return bass.AP(tensor=ap.tensor, offset=ap.offset, ap=[[0, parts], [1, free]])


def _body(ctx: ExitStack, tc: tile.TileContext, io: dict, hw: int = HW):
    nc = tc.nc
    nchunk = hw // F

    x_ap, p_ap, out_ap = io["x"], io["p"], io["out"]
    wq_ap, bq_ap = io["Wq"], io["bq"]
    wk_ap, bk_ap = io["Wk"], io["bk"]
    wv_ap, bv_ap = io["Wv"], io["bv"]
    ww_ap, bw_ap = io["Ww"], io["bw"]

    # ---------------- pools ----------------
    # NOTE: pool address space is claimed in open order, so phase-B pools are
    # opened only after the phase-A scratch scope (wkv/ppool/scratch) closes.
    consts = ctx.enter_context(tc.tile_pool(name="consts", bufs=1))
    wpool = ctx.enter_context(tc.tile_pool(name="wpool", bufs=1))
    kvout = ctx.enter_context(tc.tile_pool(name="kvout", bufs=1))
    # PSUM banks (8): qk 2 (shared with transposes) + at 3 + den 1 + fin 2
    ps_qk = ctx.enter_context(tc.tile_pool(name="ps_qk", bufs=2, space="PSUM"))
    ps_at = ctx.enter_context(tc.tile_pool(name="ps_at", bufs=3, space="PSUM"))
    ps_den = ctx.enter_context(tc.tile_pool(name="ps_den", bufs=1, space="PSUM"))
    ps_fin = ctx.enter_context(tc.tile_pool(name="ps_fin", bufs=2, space="PSUM"))
    ps_tp = ps_qk

    # ---------------- constants ----------------
    ident = consts.tile([P, P], F32, name="ident")
    make_identity(nc, ident[:])

    # den-selector weights: slice [:, h, :] is [77, 128] with ones in column h
    # -> den matmul for head h accumulates its denominator into PSUM row h.
    # (free dim padded to 128 so the PE tile config stays 128x128)
    e_all = consts.tile([CTX, NH, P], BF16, name="e_all")
    # pair-broadcast weights: e2[p, q] = (p == q // 64); slice
    # [:, 128mc:128mc+128] maps inv_all rows (2mc, 2mc+1) onto output
    # partitions (0:64, 64:128) respectively. Rows >= 16 are all zero.
    e2_all = consts.tile([P, KC * P], F32R, name="e2_all")

    # per-cout-chunk bias columns: bq_sb[:, mc] == bq[mc*128 : (mc+1)*128]
    bq_sb = consts.tile([P, KC], F32, name="bq_sb")
    nc.sync.dma_start(out=bq_sb[:], in_=bq_ap.rearrange("(a b) -> b a", b=P))
    bk_sb = consts.tile([P, KC], F32, name="bk_sb")
    nc.sync.dma_start(out=bk_sb[:], in_=bk_ap.rearrange("(a b) -> b a", b=P))
    # bw broadcast across partitions (done once via DRAM DMA)
    bw_bc = consts.tile([P, C], F32, name="bw_bc")
    nc.sync.dma_start(out=bw_bc[:], in_=_bcast_dram(bw_ap, P, C))

    # resident weights: Wq / Ww as 8 bf16 [128, 1024] k-slices (lhsT-ready)
    wq = [wpool.tile([P, C], BF16, name=f"wq{k}", tag=f"wq{k}") for k in range(KC)]
    ww = [wpool.tile([P, C], BF16, name=f"ww{k}", tag=f"ww{k}") for k in range(KC)]

    # K^T tiles [128, 77] per (batch, c-chunk); V zero-padded [77, NH, 128]:
    # head h occupies columns 64*(h%2) .. 64*(h%2)+64 of its slice so a head
    # pair accumulates into one full [128, F] PSUM tile with 128x128 tiles.
    kT = [
        [kvout.tile([P, CTX], BF16, name=f"kT{b}_{m}", tag=f"kT{b}_{m}") for m in range(KC)]
        for b in range(B)
    ]
    v_pad = [
        kvout.tile([CTX, NH, P], BF16, name=f"vpad{b}", tag=f"vpad{b}")
        for b in range(B)
    ]

    # ---------------- phase A: consts, weight casts, K/V projections ---------
    with ExitStack() as kvctx:
        wkv = kvctx.enter_context(tc.tile_pool(name="wkv", bufs=1))
        ppool = kvctx.enter_context(tc.tile_pool(name="ppool", bufs=2))
        scratch = kvctx.enter_context(tc.tile_pool(name="scratch", bufs=1))

        # gpsimd builds the selector patterns in fp32 scratch; DVE copies
        # produce the rounded tiles the matmuls require (the gpsimd ucode
        # can't take f32r/bf16 APs).
        e_scr = scratch.tile([CTX, NH, P], F32, name="e_scr", tag="e_scr")
        nc.gpsimd.memset(e_scr[:], 1.0)
        nc.gpsimd.affine_select(
            out=e_scr[:],
            in_=e_scr[:],
            compare_op=mybir.AluOpType.is_equal,
            fill=0.0,
            base=0,
            pattern=[[1, NH], [-1, P]],  # keep only a == b (inner diagonal)
            channel_multiplier=0,
        )
        e2_scr = scratch.tile([P, KC * P], F32, name="e2_scr", tag="e2_scr")
        nc.gpsimd.memset(e2_scr[:], 1.0)
        nc.gpsimd.affine_select(
            out=e2_scr[:],
            in_=e2_scr[:],
            compare_op=mybir.AluOpType.is_ge,
            fill=0.0,
            base=0,
            pattern=[[1, KC * P]],  # keep q - 64p >= 0
            channel_multiplier=-D,
        )
        nc.gpsimd.affine_select(
            out=e2_scr[:],
            in_=e2_scr[:],
            compare_op=mybir.AluOpType.is_ge,
            fill=0.0,
            base=D - 1,
            pattern=[[-1, KC * P]],  # keep 64p + 63 - q >= 0
            channel_multiplier=D,
        )
        with nc.allow_low_precision(reason="0/1 selector weights exact in bf16"):
            nc.vector.tensor_copy(out=e_all[:], in_=e_scr[:])
            nc.vector.tensor_copy(out=e2_all[:], in_=e2_scr[:])

        # cast Wq / Ww to resident bf16 tiles through a fp32 staging buffer
        with nc.allow_low_precision(reason="bf16 weights, fp32 PSUM accum"):
            for k in range(KC):
                st = scratch.tile([P, C], F32, name="wstage", tag="wstage", bufs=2)
                nc.sync.dma_start(out=st[:], in_=wq_ap[k * P : (k + 1) * P, :])
                nc.vector.tensor_copy(out=wq[k][:], in_=st[:])
                st = scratch.tile([P, C], F32, name="wstage", tag="wstage", bufs=2)
                nc.sync.dma_start(out=st[:], in_=ww_ap[k * P : (k + 1) * P, :])
                nc.vector.tensor_copy(out=ww[k][:], in_=st[:])

        wk = []
        wv = []
        for k in range(KN):
            t = wkv.tile([P, C], F32R, name=f"wk{k}", tag=f"wk{k}")
            nc.sync.dma_start(out=t[:], in_=wk_ap[k * P : (k + 1) * P, :].bitcast(F32R))
            wk.append(t)
            t = wkv.tile([P, C], F32R, name=f"wv{k}", tag=f"wv{k}")
            nc.sync.dma_start(out=t[:], in_=wv_ap[k * P : (k + 1) * P, :].bitcast(F32R))
            wv.append(t)
        # bv broadcast across partitions, only needed during phase A
        bv_bc = scratch.tile([CTX, C], F32, name="bv_bc", tag="bv_bc")
        nc.sync.dma_start(out=bv_bc[:], in_=_bcast_dram(bv_ap, CTX, C))

        for b in range(B):
            # p[b] natural [77, 512], then PE-transpose into pT [4][128, 77]
            pnat = ppool.tile([CTX, NE], F32, name="pnat", tag="pnat", bufs=2)
            nc.sync.dma_start(out=pnat[:], in_=p_ap[b])
            pT = []
            for k in range(KN):
                ps = ps_tp.tile([P, CTX], F32, name="ps_pT", tag="qk")
                nc.tensor.transpose(ps[:], pnat[:, k * P : (k + 1) * P], ident[:CTX, :CTX])
                t = ppool.tile([P, CTX], F32R, name=f"pT{k}", tag=f"pT{k}", bufs=2)
                nc.vector.tensor_copy(out=t[:], in_=ps[:])
                pT.append(t)

            with nc.allow_low_precision(reason="bf16 K/V, fp32 PSUM accum"):
                # K^T[mc] = sum_k Wk[k,mc-slice].T @ pT[k]  (+ bk)
                for mc in range(KC):
                    ps = ps_qk.tile([P, CTX], F32, name="ps_kT", tag="qk")
                    for k in range(KN):
                        # N=77 is illegal for the fp32r fast path; plain fp32
                        # here (tiny: 32 matmuls per batch).
                        nc.tensor.matmul(
                            ps[:],
                            wk[k][:, mc * P : (mc + 1) * P].bitcast(F32),
                            pT[k][:].bitcast(F32),
                            start=(k == 0),
                            stop=(k == KN - 1),
                        )
                    nc.vector.tensor_add(
                        kT[b][mc][:], ps[:], bk_sb[:, mc : mc + 1].to_broadcast([P, CTX])
                    )

                # V natural [77, c]: lhsT = pT[k] (K=128, M=77), rhs = Wv slice
                nc.gpsimd.memset(v_pad[b][:].bitcast(mybir.dt.uint16), 0.0)
                vv = v_pad[b][:].rearrange("p (a two) c -> p a two c", two=2)
                for nb in range(C // 512):
                    ps = ps_at.tile([CTX, 512], F32, name="ps_v", tag="at")
                    for k in range(KN):
                        nc.tensor.matmul(
                            ps[:],
                            pT[k][:],
                            wv[k][:, nb * 512 : (nb + 1) * 512],
                            start=(k == 0),
                            stop=(k == KN - 1),
                        )
                    pv = ps[:].rearrange("p (a two d) -> p a two d", two=2, d=D)
                    bb = bv_bc[:, nb * 512 : (nb + 1) * 512].rearrange(
                        "p (a two d) -> p a two d", two=2, d=D
                    )
                    a0 = nb * 4
                    nc.vector.tensor_add(
                        vv[:, a0 : a0 + 4, 0, 0:D], pv[:, :, 0, :], bb[:, :, 0, :]
                    )
                    nc.vector.tensor_add(
                        vv[:, a0 : a0 + 4, 1, D:P], pv[:, :, 1, :], bb[:, :, 1, :]
                    )

    # ---------------- phase B: main loop (software-pipelined) -----------
    xpool = ctx.enter_context(tc.tile_pool(name="xpool", bufs=1))
    qpool = ctx.enter_context(tc.tile_pool(name="qpool", bufs=1))
    apool = ctx.enter_context(tc.tile_pool(name="apool", bufs=1))
    epool = ctx.enter_context(tc.tile_pool(name="epool", bufs=4))
    opool = ctx.enter_context(tc.tile_pool(name="opool", bufs=4))
    spool = ctx.enter_context(tc.tile_pool(name="spool", bufs=4))
    ctx.enter_context(
        nc.allow_low_precision(reason="bf16 activations, fp32 PSUM accum")
    )

    chunks = [(b, j) for b in range(B) for j in range(nchunk)]

    def emit_xn(idx):
        """Issue the x DMA loads for chunk `idx` (4 natural [128, 1024])."""
        b, j = chunks[idx]
        r0 = j * F
        xn = []
        for r in range(FSUB):
            t = xpool.tile([P, C], F32, name="xn", tag="xn", bufs=6)
            nc.sync.dma_start(
                out=t[:], in_=x_ap[b, r0 + r * P : r0 + (r + 1) * P, :]
            )
            xn.append(t)
        return xn

    def emit_T(xn):
        """PE-transpose xn -> xT[kc] [128(c), F(hw)] bf16 (copies on DVE)."""
        xT = []
        for kc in range(KC):
            ps = ps_tp.tile([P, F], F32, name="ps_xT", tag="qk")
            for r in range(FSUB):
                nc.tensor.transpose(
                    ps[:, r * P : (r + 1) * P],
                    xn[r][:, kc * P : (kc + 1) * P],
                    ident[:],
                )
            t = xpool.tile([P, F], BF16, name="xT", tag="xT", bufs=10)
            nc.vector.tensor_copy(out=t[:], in_=ps[:])
            xT.append(t)
        return xT

    # preamble: first chunk's loads + transposes
    xT_cur = emit_T(emit_xn(0))

    for idx, (b, j) in enumerate(chunks):
        r0 = j * F
        xn_next = emit_xn(idx + 1) if idx + 1 < len(chunks) else None

        # Q^T[mc] = sum_kc Wq[kc, mc-slice].T @ xT[kc]  (+ bq on ScalarE)
        qT = []
        for mc in range(KC):
            ps = ps_qk.tile([P, F], F32, name="ps_qT", tag="qk")
            for kc in range(KC):
                nc.tensor.matmul(
                    ps[:],
                    wq[kc][:, mc * P : (mc + 1) * P],
                    xT_cur[kc][:],
                    start=(kc == 0),
                    stop=(kc == KC - 1),
                )
            t = qpool.tile([P, F], BF16, name="qT", tag="qT", bufs=10)
            nc.scalar.add(t[:], ps[:], bq_sb[:, mc : mc + 1])
            qT.append(t)

        # scores per head into separate PSUM tiles (two independent matmul
        # groups must not share a PSUM bank on hw); exp lands in the pair
        # SBUF tile so den/av consume [77, 2, F] slices.
        exs = []
        for mc in range(KC):
            ex = epool.tile([CTX, 2, F], BF16, name="ex", tag="ex", bufs=8)
            for half in range(2):
                ps_s = ps_at.tile([CTX, F], F32, name="ps_s", tag="at")
                nc.tensor.matmul(
                    ps_s[:],
                    kT[b][mc][half * D : (half + 1) * D, :],
                    qT[mc][half * D : (half + 1) * D, :],
                    start=True,
                    stop=True,
                    tile_position=(half * D, 0),
                )
                nc.scalar.activation(
                    ex[:, half, :],
                    ps_s[:],
                    mybir.ActivationFunctionType.Exp,
                    scale=0.125,
                )
            exs.append(ex)

        # all 16 denominators into rows 0:16 of one PSUM tile (accumulating
        # selector matmuls), then a single reciprocal for the chunk
        den = ps_den.tile([P, F], F32, name="den", tag="den")
        for h in range(NH):
            nc.tensor.matmul(
                den[:],
                e_all[:, h, :],
                exs[h // 2][:, h % 2, :],
                start=(h == 0),
                stop=(h == NH - 1),
            )
        inv_all = spool.tile([P, F], F32R, name="inv_all", tag="inv", bufs=2)
        nc.gpsimd.memset(inv_all[:].bitcast(F32), 0.0)
        nc.vector.reciprocal(out=inv_all[0:NH, :], in_=den[0:NH, :])

        # next chunk's transposes keep the PE busy while the reciprocal runs
        xT_next = emit_T(xn_next) if xn_next is not None else None

        aT = [
            apool.tile([P, F], BF16, name="aT", tag="aT", bufs=10)
            for _ in range(KC)
        ]
        for mc in range(KC):
            # attn-out^T pair [128, F]: rows 0:64 head 2mc, 64:128 head 2mc+1
            # (zero-padded V slices -> both halves accumulate in one tile)
            ps_av = ps_at.tile([P, F], F32, name="ps_av", tag="at")
            for half in range(2):
                h = 2 * mc + half
                nc.tensor.matmul(
                    ps_av[:],
                    v_pad[b][:, h, :],
                    exs[mc][:, half, :],
                    start=(half == 0),
                    stop=(half == 1),
                )
            # inv rows (2mc, 2mc+1) broadcast onto partitions via PE
            ps_bc = ps_at.tile([P, F], F32, name="ps_bc", tag="at")
            nc.tensor.matmul(
                ps_bc[:],
                e2_all[:, P * mc : P * (mc + 1)],
                inv_all[:],
                start=True,
                stop=True,
            )
            bc = spool.tile([P, F], F32, name="bc", tag="bc")
            nc.scalar.copy(bc[:], ps_bc[:])
            nc.vector.tensor_mul(aT[mc][:], ps_av[:], bc[:])

        # final projection, natural orientation: out[hw128, c]
        for fs in range(FSUB):
            osb = opool.tile([P, C], F32, name="osb", tag="osb", bufs=3)
            for nb in range(C // 512):
                ps = ps_fin.tile([P, 512], F32, name="ps_f", tag="fin")
                for kc in range(KC):
                    nc.tensor.matmul(
                        ps[:],
                        aT[kc][:, fs * P : (fs + 1) * P],
                        ww[kc][:, nb * 512 : (nb + 1) * 512],
                        start=(kc == 0),
                        stop=(kc == KC - 1),
                    )
                nc.vector.tensor_add(
                    osb[:, nb * 512 : (nb + 1) * 512],
                    ps[:],
                    bw_bc[:, nb * 512 : (nb + 1) * 512],
                )
            nc.sync.dma_start(
                out=out_ap[b, r0 + fs * P : r0 + (fs + 1) * P, :], in_=osb[:]
            )

        xT_cur = xT_next


def build_program(hw: int = HW):
    """Build + compile the per-core Bass program (SPMD, identical per core)."""
    nc = bacc.Bacc(
        "TRN2", target_bir_lowering=False, debug=False, num_devices=N_CORES
    )
    io = {}
    io["x"] = nc.dram_tensor("x", [B, hw, C], F32, kind="ExternalInput").ap()
    io["p"] = nc.dram_tensor("p", [B, CTX, NE], F32, kind="ExternalInput").ap()
    for name, shape in [
        ("Wq", [C, C]),
        ("bq", [C]),
        ("Wk", [NE, C]),
        ("bk", [C]),
        ("Wv", [NE, C]),
        ("bv", [C]),
        ("Ww", [C, C]),
        ("bw", [C]),
    ]:
        io[name] = nc.dram_tensor(name, shape, F32, kind="ExternalInput").ap()
    io["out"] = nc.dram_tensor("out", [B, hw, C], F32, kind="ExternalOutput").ap()

    with tile.TileContext(nc) as tc:
        with ExitStack() as ctx:
            _body(ctx, tc, io, hw=hw)
    nc.compile()
    return nc


_PROGRAM = None


def run_sharded(inputs: dict, trace: bool = False, **trace_kwargs):
    """Shard inputs over the 8 cores, run, gather. Returns (out, results)."""
    global _PROGRAM
    if _PROGRAM is None:
        _PROGRAM = build_program()
    nc = _PROGRAM

    full = {
        k: np.ascontiguousarray(v, dtype=np.float32)
        for k, v in inputs.items()
    }
    in_maps = []
    for i in range(N_CORES):
        m = dict(full)
        m["x"] = full["x"][i * B : (i + 1) * B]
        m["p"] = full["p"][i * B : (i + 1) * B]
        in_maps.append(m)

    res = run_bass_kernel_spmd(
        nc, in_maps, list(range(N_CORES)), trace=trace, **trace_kwargs
    )
    out = np.concatenate([res.results[i]["out"] for i in range(N_CORES)], axis=0)
    return out, res


def kernel(x, p, Wq, bq, Wk, bk, Wv, bv, Ww, bw):
    out, _ = run_sharded(
        dict(x=x, p=p, Wq=Wq, bq=bq, Wk=Wk, bk=bk, Wv=Wv, bv=bv, Ww=Ww, bw=bw)
    )
    return out
